# revision 49
# baseline (speedup 1.0000x reference)
"""DynamicConvolution TRN2 Bass kernel.

Problem (per reference):
  x: (32, 128, 64, 64) f32
  attention: pooled = mean(x, HW) -> MLP (relu) -> prompt dot -> softmax over K=8
  agg_w[b] = sum_k alpha[b,k] * kernels_weights[k]  (K=8 banks of (128,128,3,3))
  out[b] = conv2d(x[b], agg_w[b], pad=1) + agg_b[b]   -> (32, 128, 64, 64)

Strategy (steady-state ~63us predicted / ~74us measured per exec per core,
vs the 61.4us fp16 PE roofline):
  - Data-parallel over batch: 8 cores x 4 samples, no collectives.
  - Conv as 9 shifted matmuls accumulating in PSUM, fp16 operands
    (1 col/cycle on the PE), tap-major over 4-chunk PSUM groups so each
    stationary tile feeds 4x512 moving columns.
  - x / kernel bank / MLP weights pre-cast to fp16 on host (halves DMA and
    doubles DVE throughput; ~5e-4 rel error, 40x inside the 2e-2 gate).
  - Per-sample attention pipelined into the conv stream: sample 0's MLP
    gates only the first conv group; later samples' reduces (quartered
    across DVE/ACT), MLPs and aggregations run between conv groups,
    emitted with tc.high_priority so the Tile scheduler keeps them off
    the PE's critical path.
  - Softmax without max-subtract (scores are O(1) here), exp+sum fused in
    one ACT op; alphas broadcast to 128 partitions via ones-vector
    matmuls on the PE (no DRAM round trip).
  - Bank aggregation: serial DVE scalar_tensor_tensor chains over k-major
    contiguous slices (TensorScalarPtr is illegal on Pool/GPSIMD);
    sample 0's chain is split in two tap-ranges pipelined with the 4-piece
    kw DMA.
  - Per-chunk output stores; final group chunk-major with a split last
    chunk so the tail evict+store chain is minimal.
  - build(reps=N) unrolls the whole pipeline for clean on-device timing.
"""
import sys

sys.path.insert(0, "/opt/trn_rl_repo")

import numpy as np

import concourse.bacc as bacc
import concourse.mybir as mybir
import concourse.tile as tile
from concourse.bass_utils import run_bass_kernel_spmd

# problem dims
B, C, H, W = 32, 128, 64, 64
K, KS = 8, 3
NTAP = KS * KS
HID = 512
NCORES = 8
BL = B // NCORES          # local batch = 4
HP, WP = H + 2, W + 2     # 66x66 padded
NPIX = HP * WP            # 4356
RCHUNK = 8                # output rows per PSUM chunk
NCHUNK = H // RCHUNK      # 8
GCH = 4                   # chunks per PSUM group
F32 = mybir.dt.float32
F16 = mybir.dt.float16
AF = mybir.ActivationFunctionType
ALU = mybir.AluOpType


def build(timing_chain: bool = False, probe_skip=(), reps: int = 1):
    nc = bacc.Bacc("TRN2", target_bir_lowering=False, debug=False)

    if timing_chain:
        nc.dram_tensor("chain", [BL, C, H * W], F32, kind="ExternalInput")
    xp = nc.dram_tensor("xp", [BL, C, NPIX], F16, kind="ExternalInput")
    # wpack = w1t (512) | w2t (4*512) | pt (4*8), all fp16, per-C row
    wpack = nc.dram_tensor("wpack", [C, HID + 4 * HID + 4 * K], F16,
                           kind="ExternalInput")
    bpack = nc.dram_tensor("bpack", [C, 8], F32, kind="ExternalInput")
    kb = nc.dram_tensor("kb", [K, C], F32, kind="ExternalInput")
    kw = nc.dram_tensor("kw", [C, K, NTAP, C], F16, kind="ExternalInput")
    out = nc.dram_tensor("out", [BL, C, H * W], F32, kind="ExternalOutput")

    taps = [(ti, tj) for ti in range(KS) for tj in range(KS)]

    with tile.TileContext(nc) as tc:
        with (
            tc.tile_pool(name="singles", bufs=1) as singles,
            tc.tile_pool(name="xpool", bufs=BL) as xpool,
            tc.tile_pool(name="opool", bufs=4) as opool,
            tc.tile_pool(name="aggpool", bufs=2) as aggpool,
            tc.tile_pool(name="scr", bufs=4) as scr,
            tc.tile_pool(name="psum", bufs=1, space="PSUM") as psum,
        ):
            # ---- t=0: preload ACT table (Exp set) so it's off the MLP path
            scrap = singles.tile([1, 1], F32)
            nc.gpsimd.memset(scrap, 0.0)
            scrap2 = singles.tile([1, 1], F32)
            nc.scalar.activation(scrap2, scrap, AF.Exp)
            # ones row for partition-broadcast matmuls
            ones1 = singles.tile([1, C], F32)
            nc.gpsimd.memset(ones1, 1.0)

            # ---- persistent weight tiles (DMAs re-issued per rep) ----
            # uneven: the last piece is small so the final quarter-reduce
            # (which gates pooled) is short
            QROWS = [(0, 33), (33, HP)]
            x_sb = []
            wpack_sb = singles.tile([C, HID + 4 * HID + 4 * K], F16)
            bpack_sb = singles.tile([C, 8], F32)
            kb_sb = singles.tile([K, C], F32)
            kw_sb = singles.tile([C, K, NTAP, C], F16)
            w1t_sb = wpack_sb[:, 0:HID]
            w2t_sb = wpack_sb[:, HID:5 * HID].rearrange("p (a b) -> p a b", a=4)
            pt_sb = wpack_sb[:, 5 * HID:].rearrange("p (a b) -> p a b", a=4)
            b1_sb = bpack_sb[:, 0:4]
            b2_sb = bpack_sb[:, 4:8]

            def load_dmas():
                """All input DMAs, in latency-priority order."""
                x_sb.clear()
                for s in range(BL):
                    xt = xpool.tile([C, HP, WP], F16, tag="x", name=f"x{s}")
                    x_sb.append(xt)
                # sample 0 in four pieces so pooled can start early
                xv0 = xp.ap()[0].rearrange("p (a b) -> p a b", a=HP)
                for r0, r1 in QROWS:
                    nc.sync.dma_start(
                        out=x_sb[0][:, r0:r1, :], in_=xv0[:, r0:r1, :]
                    )
                nc.sync.dma_start(out=wpack_sb, in_=wpack.ap())
                nc.sync.dma_start(out=bpack_sb, in_=bpack.ap())
                nc.sync.dma_start(out=kb_sb, in_=kb.ap())
                # kernel bank, k-major (contiguous per-k slices for the DVE
                # aggregation chains), 2 banks per DMA piece
                for kgrp in range(2):
                    nc.sync.dma_start(
                        out=kw_sb[:, 4 * kgrp:4 * (kgrp + 1)],
                        in_=kw.ap()[:, 4 * kgrp:4 * (kgrp + 1)],
                    )
                for s in range(1, BL):
                    nc.sync.dma_start(
                        out=x_sb[s],
                        in_=xp.ap()[s].rearrange("p (a b) -> p a b", a=HP),
                    )

            # ---- persistent small tiles ----
            pooled = singles.tile([C, BL], F16)       # per-sample pixel sums
            pooledf = singles.tile([C, BL], F32)      # fp32 reduce staging
            junk = singles.tile([C, NPIX], F16)       # ACT accum side-output
            h_sb = singles.tile([C, 4, BL], F16)
            s_sb = singles.tile([C, 4, BL], F16)
            albc_sb = singles.tile([C, BL, K], F32)   # alphas bcast to 128 parts
            alk8_sb = singles.tile([K, BL], F32)      # alphas with k on partitions
            aggb_sb = singles.tile([C, BL], F32)      # aggregated conv bias

            def reduce_full(s):
                """Pixel-sum of sample s in 4 quarter ops (DVE/ACT alternate)
                so no single blocky op can delay the MLP chains."""
                ctx = tc.high_priority()
                ctx.__enter__()
                red4 = scr.tile([C, 4], F32, tag="red4", name=f"red4_{s}")
                for qi, (r0, r1) in enumerate(QROWS):
                    if qi % 2 == 0:
                        nc.vector.tensor_reduce(
                            red4[:, qi:qi + 1], x_sb[s][:, r0:r1, :],
                            axis=mybir.AxisListType.XY, op=ALU.add,
                        )
                    else:
                        nc.scalar.activation(
                            junk[:, 0:(r1 - r0) * WP],
                            x_sb[s][:, r0:r1, :].rearrange("p a b -> p (a b)"),
                            AF.Copy, accum_out=red4[:, qi:qi + 1],
                        )
                nc.vector.tensor_reduce(
                    pooledf[:, s:s + 1], red4[:, 0:len(QROWS)],
                    axis=mybir.AxisListType.X, op=ALU.add,
                )
                nc.scalar.copy(pooled[:, s:s + 1], pooledf[:, s:s + 1])
                ctx.__exit__(None, None, None)

            def mlp(s, ctx_prio=True):
                """Attention MLP + softmax + alpha broadcasts for sample s.
                Emitted with high priority: the whole chain is tiny but sits
                on the critical path to the next sample's aggregation."""
                import contextlib
                prio = tc.high_priority() if ctx_prio else contextlib.nullcontext()
                with prio:
                    _mlp_body(s)

            def _mlp_body(s):
                sl = slice(s, s + 1)
                ps_h = psum.tile([C, 4, 1], F32, tag="ps_small", bufs=2)
                for c in range(4):
                    nc.tensor.matmul(
                        ps_h[:, c, :], w1t_sb[:, 128 * c:128 * (c + 1)],
                        pooled[:, sl], start=True, stop=True,
                    )
                for c in range(4):
                    nc.scalar.activation(
                        h_sb[:, c, sl], ps_h[:, c, :], AF.Relu,
                        bias=b1_sb[:, c:c + 1], scale=1.0 / (H * W),
                    )
                ps_s = psum.tile([C, 4, 1], F32, tag="ps_small", bufs=2)
                for c2 in range(4):
                    for c in range(4):
                        nc.tensor.matmul(
                            ps_s[:, c2, :],
                            w2t_sb[:, c, 128 * c2:128 * (c2 + 1)],
                            h_sb[:, c, sl],
                            start=(c == 0), stop=(c == 3),
                        )
                for c2 in range(4):
                    nc.scalar.activation(
                        s_sb[:, c2, sl], ps_s[:, c2, :], AF.Identity,
                        bias=b2_sb[:, c2:c2 + 1],
                    )
                ps_sc = psum.tile([1, K], F32, tag="ps_small", bufs=2)
                for c2 in range(4):
                    nc.tensor.matmul(
                        ps_sc, s_sb[:, c2, sl], pt_sb[:, c2, :],
                        start=(c2 == 0), stop=(c2 == 3),
                    )
                # scores here are O(1): exp cannot overflow, so skip the
                # max-subtract and fuse the sum into the exp's accumulator
                ex = scr.tile([1, K], F32, tag="ex")
                sm = scr.tile([1, 1], F32, tag="sm")
                nc.scalar.activation(ex, ps_sc, AF.Exp, accum_out=sm)
                rsm = scr.tile([1, 1], F32, tag="rsm")
                nc.vector.reciprocal(rsm, sm)
                alphas_s = scr.tile([1, K], F32, tag="alphas")
                nc.vector.tensor_scalar_mul(alphas_s, ex, rsm)

                # broadcast alpha row to all 128 partitions: ones^T @ alphas
                ps_bc = psum.tile([C, K], F32, tag="ps_small", bufs=2)
                nc.tensor.matmul(
                    ps_bc, ones1, alphas_s, start=True, stop=True
                )
                nc.scalar.copy(albc_sb[:, s, :], ps_bc)
                # alphas with k on partitions: alphas^T @ [1]
                ps_k8 = psum.tile([K, 1], F32, tag="ps_small", bufs=2)
                nc.tensor.matmul(
                    ps_k8, alphas_s, ones1[:, 0:1], start=True, stop=True
                )
                nc.scalar.copy(alk8_sb[:, sl], ps_k8)
                # aggregated bias: kb^T @ alpha
                ps_ab = psum.tile([C, 1], F32, tag="ps_small", bufs=2)
                nc.tensor.matmul(
                    ps_ab, kb_sb, alk8_sb[:, sl], start=True, stop=True
                )
                nc.scalar.copy(aggb_sb[:, sl], ps_ab)

            def new_aggw(s):
                return aggpool.tile(
                    [C, NTAP, C], F16, tag="aggw", bufs=4, name=f"aggw{s}"
                )

            def agg_chain(s, tap_splits):
                """Aggregate the bank for sample s: per tap-range, a DVE
                chain over the 8 banks (contiguous k-major slices).  Chain
                op k gates only on kw DMA piece k//2, so sample 0's chain
                pipelines with the kw load."""
                al = albc_sb[:, s, :]
                aggw = new_aggw(s)
                ctx = tc.high_priority()
                ctx.__enter__()
                for g0, g1 in tap_splits:
                    nt = g1 - g0
                    pa = [
                        aggpool.tile([C, NTAP, C], F16, tag="pa",
                                     name=f"pa{i}")[:, 0:nt, :]
                        for i in range(2)
                    ]
                    kws = [kw_sb[:, k, g0:g1, :] for k in range(K)]
                    nc.vector.tensor_scalar_mul(pa[0], kws[0], al[:, 0:1])
                    for i, k in enumerate(range(1, K - 1)):
                        nc.vector.scalar_tensor_tensor(
                            pa[(i + 1) % 2], kws[k], al[:, k:k + 1], pa[i % 2],
                            op0=ALU.mult, op1=ALU.add,
                        )
                    nc.vector.scalar_tensor_tensor(
                        aggw[:, g0:g1, :], kws[K - 1], al[:, K - 1:K],
                        pa[(K - 2) % 2],
                        op0=ALU.mult, op1=ALU.add,
                    )
                ctx.__exit__(None, None, None)
                return aggw

            def mm_group(s, g, aggw):
                """Matmuls for chunks 4g..4g+3 of sample s, tap-major."""
                ps = [
                    psum.tile([C, RCHUNK, W], F32, tag="ps_c", bufs=6,
                              name=f"ps_c{i}")
                    for i in range(GCH)
                ]
                for t, (ti, tj) in enumerate(taps):
                    for ci in range(GCH):
                        h0 = (GCH * g + ci) * RCHUNK
                        nc.tensor.matmul(
                            ps[ci], aggw[:, t, :],
                            x_sb[s][:, h0 + ti:h0 + ti + RCHUNK, tj:tj + W],
                            start=(t == 0), stop=(t == NTAP - 1),
                        )
                return ps

            def evict_group(s, g, ps):
                """PSUM -> SBUF (+bias) on ACT; chunk-pair stores (half the
                DMA instructions on the store path)."""
                for pi in range(GCH // 2):
                    o2 = opool.tile([C, 2 * RCHUNK, W], F32, tag="o",
                                    name=f"o{pi}")
                    for hi in range(2):
                        ci = 2 * pi + hi
                        nc.scalar.activation(
                            o2[:, hi * RCHUNK:(hi + 1) * RCHUNK, :], ps[ci],
                            AF.Identity, bias=aggb_sb[:, s:s + 1]
                        )
                    c = GCH * g + 2 * pi
                    nc.sync.dma_start(
                        out=out.ap()[s][:, c * RCHUNK * W:(c + 2) * RCHUNK * W],
                        in_=o2.rearrange("p a b -> p (a b)"),
                    )

            def last_group(s, g, aggw):
                """Final conv group: chunk-major so chunks finish (and store)
                progressively; the very last chunk is split in half rows so
                the tail evict+store chain is as short as possible."""
                for ci in range(GCH):
                    c = GCH * g + ci
                    h0 = c * RCHUNK
                    pc = psum.tile([C, RCHUNK, W], F32, tag="ps_c", bufs=6,
                                   name=f"ps_c{ci}")
                    halves = [(0, RCHUNK)] if ci < GCH - 1 else \
                        [(0, RCHUNK // 2), (RCHUNK // 2, RCHUNK)]
                    for r0, r1 in halves:
                        for t, (ti, tj) in enumerate(taps):
                            nc.tensor.matmul(
                                pc[:, r0:r1, :], aggw[:, t, :],
                                x_sb[s][:, h0 + r0 + ti:h0 + r0 + ti + (r1 - r0),
                                        tj:tj + W],
                                start=(t == 0), stop=(t == NTAP - 1),
                            )
                        o = opool.tile([C, RCHUNK, W], F32, tag="o",
                                       name=f"o{ci}")[:, r0:r1, :]
                        nc.scalar.activation(
                            o, pc[:, r0:r1, :], AF.Identity,
                            bias=aggb_sb[:, s:s + 1]
                        )
                        nc.sync.dma_start(
                            out=out.ap()[s][:, (c * RCHUNK + r0) * W:
                                            (c * RCHUNK + r1) * W],
                            in_=o.rearrange("p a b -> p (a b)"),
                        )

            # ---- schedule (one full execution; repeated `reps` times for
            # the timing build — iterations pipeline via tile-ring deps) ----
            def body():
                load_dmas()
                reduce_full(0)
                mlp(0)
                aggw0 = agg_chain(0, [(0, 5), (5, 9)])
                ps = mm_group(0, 0, aggw0)
                reduce_full(1)
                evict_group(0, 0, ps)
                mlp(1)
                aggw1 = agg_chain(1, [(0, 9)])
                ps = mm_group(0, 1, aggw0)
                reduce_full(2)
                evict_group(0, 1, ps)
                mlp(2)
                aggw2 = agg_chain(2, [(0, 9)])
                ps = mm_group(1, 0, aggw1)
                reduce_full(3)
                evict_group(1, 0, ps)
                mlp(3)
                aggw3 = agg_chain(3, [(0, 9)])
                ps = mm_group(1, 1, aggw1)
                evict_group(1, 1, ps)
                ps = mm_group(2, 0, aggw2)
                evict_group(2, 0, ps)
                ps = mm_group(2, 1, aggw2)
                evict_group(2, 1, ps)
                ps = mm_group(3, 0, aggw3)
                evict_group(3, 0, ps)
                last_group(3, 1, aggw3)

            for _ in range(reps):
                body()

    nc.compile()
    return nc


_NC = None


def _get_nc():
    global _NC
    if _NC is None:
        _NC = build()
    return _NC


def prep_inputs(x, prompt_param, w1, b1, w2, b2, kernels_weights, kernels_bias):
    """Host-side layout transforms -> per-core in_maps."""
    x = np.asarray(x, np.float32)
    prompt = np.asarray(prompt_param, np.float32)[0]          # (K, HID)
    w1 = np.asarray(w1, np.float32)
    b1 = np.asarray(b1, np.float32)
    w2 = np.asarray(w2, np.float32)
    b2 = np.asarray(b2, np.float32)
    kwt = np.asarray(kernels_weights, np.float32)             # (K, O, I, 3, 3)
    kbt = np.asarray(kernels_bias, np.float32)                # (K, C)

    w1t = np.ascontiguousarray(w1.T)                          # (C, HID)
    w2t = w2.T.reshape(4, C, HID).transpose(1, 0, 2)          # (C, 4, HID)
    pt = prompt.T.reshape(4, C, K).transpose(1, 0, 2)         # (C, 4, K)
    wpack = np.concatenate(
        [w1t.reshape(C, HID), w2t.reshape(C, 4 * HID), pt.reshape(C, 4 * K)],
        axis=1,
    ).astype(np.float16)
    bpack = np.concatenate(
        [b1.reshape(4, C).T, b2.reshape(4, C).T], axis=1
    ).astype(np.float32)
    bpack = np.ascontiguousarray(bpack)
    # k-major kernel bank: (I, K, kh, kw, O)
    kwl = np.ascontiguousarray(
        kwt.transpose(2, 0, 3, 4, 1).reshape(C, K, NTAP, C)
    ).astype(np.float16)
    kbl = np.ascontiguousarray(kbt)

    in_maps = []
    for c in range(NCORES):
        xs = x[c * BL:(c + 1) * BL]                            # (4, C, H, W)
        xpad = np.zeros((BL, C, HP, WP), np.float16)
        xpad[:, :, 1:H + 1, 1:W + 1] = xs
        in_maps.append(
            {
                "xp": xpad.reshape(BL, C, NPIX), "wpack": wpack,
                "bpack": bpack, "kb": kbl, "kw": kwl,
            }
        )
    return in_maps


def kernel(**inputs) -> np.ndarray:
    nc = _get_nc()
    in_maps = prep_inputs(**inputs)
    res = run_bass_kernel_spmd(nc, in_maps, core_ids=list(range(NCORES)))
    outs = [res.results[c]["out"].reshape(BL, C, H, W) for c in range(NCORES)]
    return np.concatenate(outs, axis=0)


if __name__ == "__main__":
    import reference

    inputs = {k: np.asarray(v) for k, v in reference.setup_inputs().items()}
    expected = np.asarray(reference.reference(**inputs))
    actual = kernel(**inputs)
    scale = np.abs(expected).max()
    err = np.abs(actual - expected).max()
    print(f"absmax={err:.3e} scale={scale:.3f} rel={err / scale:.3e}")
